# revision 23
# baseline (speedup 1.0000x reference)
"""GRUCell + LayerNorm readout fused Bass kernel for Trainium2 (8 NeuronCores).

Problem: B=8192, D=H=O=1024 fp32.
    r = sigmoid(x@Wir + bir + h@Whr)
    z = sigmoid(x@Wiz + biz + h@Whz)
    n = tanh(x@Win + bin_ + r*(h@Whn + bhn))
    new_h = (1-z)*n + z*h
    out = (LayerNorm(new_h)*ln_scale + ln_bias) @ Wout + bout

Strategy:
  - Data-parallel over batch: core c gets rows [c*1024, (c+1)*1024); weights
    replicated. No collectives.
  - Everything computed in the transposed domain: activations live as
    [feature, batch] so matmuls take the weights in natural [k, h] layout as
    the stationary operand and xT/hT as the moving operand, and the per-h gate
    biases become per-partition activation biases. Host passes xT/hT and
    transposes the outputs back.
  - All matmul operands are bf16: same PE rate as f32r (1 col/cycle) but half
    the HBM traffic, and LDWEIGHTS gets the fast-weight-load path that
    fp32-wide dtypes can't use. Outputs (new_h/out) are stored bf16 and
    widened on host. Weights are pre-transposed on host into the exact SBUF
    layout so every DMA is a flat 2D transfer (cheap descriptor generation).
  - Ramp: input tiles stream in 2-k-slice chunks alternating between the
    gpsimd and scalar DMA rings with weights on the sync ring, ordered to
    match consumption (each ring sustains ~105 GB/s independently).
    Per (ht, bc) the gates run x-side chains first (r, z, gi), then h-side
    (r, z, gh), so the first half of each block only needs the x tiles.
    Contiguous accumulation chains stagger PSUM bank frees so the next
    block's start=True matmuls never stall on epilogue consumers.
  - LayerNorm reduces over h (= partition dim): per-tile partials accumulate
    elementwise on DVE, one ones-column matmul per stat at the end.
    The normalize-then-matmul is algebraically folded:
        LN(new_h) @ (ln_scale*Wout) + (ln_bias@Wout + bout)
      = rstd[b]*( new_h@WoutF - mu[b]*colsum[o] ) + boutF[o]
    with WoutF = ln_scale[:,None]*Wout (host), colsum = ln_scale@Wout (host);
    the mu*colsum term is a K=1 rank-1 matmul closing each readout group's
    PSUM accumulation and the +boutF[o] bias is added on host after gather.
"""

import sys
from contextlib import ExitStack

sys.path.insert(0, "/opt/trn_rl_repo")

import ml_dtypes
import numpy as np

import concourse.bacc as bacc
import concourse.mybir as mybir
import concourse.tile as tile
from concourse import bass_utils

B, D, H, O = 8192, 1024, 1024, 1024
NCORES = 8
BL = B // NCORES          # batch rows per core
P = 128                   # partitions
KT = D // P               # contraction tiles (8)
HT = H // P               # h output-partition tiles (8)
OT = O // P               # o output-partition tiles (8)
NB = 2                    # batch chunks per core (free dim 512)
NF = BL // NB             # free dim per chunk (512)
LN_EPS = 1e-6

F32 = mybir.dt.float32
F32R = mybir.dt.float32r
BF16 = mybir.dt.bfloat16
BF16NP = ml_dtypes.bfloat16

_COMPILED = None  # compiled Bacc module cache across calls
TRACE = False     # set by test harness to capture an NTFF profile
LAST_RES = None   # BassKernelResults of the last run (for the test harness)

XGATES = ("ir", "iz", "in")
HGATES = ("hr", "hz", "hn")


def _build():
    nc = bacc.Bacc("TRN2", target_bir_lowering=False, debug=False,
                   num_devices=NCORES)

    def din(name, shape, dt=BF16):
        return nc.dram_tensor(name, shape, dt, kind="ExternalInput").ap()

    xT_d = din("xT", [D, BL])
    hT_d = din("hT", [H, BL])
    # weights pre-laid-out on host: w[p, ht*KT*P + k*P + j] = W[k*P+p, ht*P+j]
    w_d = {g: din(f"W{g}", [P, HT * KT * P]) for g in XGATES + HGATES}
    woutF_d = din("woutF", [P, KT * O])
    bir_d = din("bir", [P, HT], F32)
    biz_d = din("biz", [P, HT], F32)
    bin_d = din("bin", [P, HT], F32)
    bhn_d = din("bhn", [P, HT], F32)
    colsum_d = din("colsum", [1, O])
    ones_col_d = din("ones_col", [P, 1], F32R)
    ones_row_d = din("ones_row", [1, P], F32R)

    nhT_d = nc.dram_tensor("nhT", [H, BL], BF16, kind="ExternalOutput").ap()
    outT_d = nc.dram_tensor("outT", [O, BL], BF16, kind="ExternalOutput").ap()

    with tile.TileContext(nc) as tc, ExitStack() as ctx:
        singles = ctx.enter_context(tc.tile_pool(name="singles", bufs=1))
        wpool = ctx.enter_context(tc.tile_pool(name="wpool", bufs=2))
        gates = ctx.enter_context(tc.tile_pool(name="gates", bufs=1))
        rows = ctx.enter_context(tc.tile_pool(name="rows", bufs=1))
        ps = ctx.enter_context(tc.tile_pool(name="ps", bufs=1, space="PSUM"))

        # ---- resident input tiles --------------------------------------------
        # One [P, KT, NF] tile per (side, bc), loaded in few big chunked 3D
        # DMAs (per-ring throughput is trigger-rate * transfer-size, so big
        # transfers matter): x on the gpsimd ring, h on the scalar ring,
        # weights on sync — three parallel streams whose arrival order
        # matches consumption.
        xb = [singles.tile([P, KT, NF], BF16, tag=f"x{bc}", name=f"x{bc}")
              for bc in range(NB)]
        hb = [singles.tile([P, KT, NF], BF16, tag=f"h{bc}", name=f"h{bc}")
              for bc in range(NB)]

        def wtile(g, ht):
            return wpool.tile([P, KT * P], BF16, tag=f"w{g}", name=f"w{g}_{ht}")

        def load_w_flat(t, g, ht):
            nc.sync.dma_start(t[:], w_d[g][:, ht * KT * P:(ht + 1) * KT * P])

        def load_in(eng, t, src_d, bc, k0, nk):
            eng.dma_start(
                t[:, k0:k0 + nk, :],
                src_d[k0 * P:(k0 + nk) * P,
                      bc * NF:(bc + 1) * NF].rearrange("(t p) b -> p t b",
                                                       p=P))

        w0 = {g: wtile(g, 0) for g in XGATES + HGATES}
        # wir in two halves so the first matmul waits on 128KB, not 256KB
        nc.sync.dma_start(w0["ir"][:, :KT * P // 2],
                          w_d["ir"][:, :KT * P // 2])
        nc.sync.dma_start(w0["ir"][:, KT * P // 2:],
                          w_d["ir"][:, KT * P // 2:KT * P])
        # bc0 chunks alternate between the gpsimd and scalar rings so both
        # rings deliver the x tiles (then the h tiles) in parallel, in
        # consumption order; each ring runs ~105 GB/s regardless of size.
        load_in(nc.gpsimd, xb[0], xT_d, 0, 0, 2)
        load_in(nc.scalar, xb[0], xT_d, 0, 2, 2)
        load_in(nc.gpsimd, xb[0], xT_d, 0, 4, 2)
        load_in(nc.scalar, xb[0], xT_d, 0, 6, 2)
        load_w_flat(w0["iz"], "iz", 0)
        load_in(nc.scalar, hb[0], hT_d, 0, 0, 2)
        load_in(nc.gpsimd, hb[0], hT_d, 0, 2, 2)
        load_in(nc.scalar, hb[0], hT_d, 0, 4, 2)
        load_in(nc.gpsimd, hb[0], hT_d, 0, 6, 2)
        load_w_flat(w0["in"], "in", 0)
        load_w_flat(w0["hr"], "hr", 0)
        load_w_flat(w0["hz"], "hz", 0)
        load_w_flat(w0["hn"], "hn", 0)
        for k0 in range(0, KT, 4):
            load_in(nc.gpsimd, xb[1], xT_d, 1, k0, 4)
        for k0 in range(0, KT, 4):
            load_in(nc.scalar, hb[1], hT_d, 1, k0, 4)

        def load_vec(ap_d, tag):
            t = singles.tile([P, HT], F32, tag=tag, name=tag)
            nc.sync.dma_start(t[:], ap_d)
            return t

        bir_sb = load_vec(bir_d, "bir_sb")
        biz_sb = load_vec(biz_d, "biz_sb")
        bin_sb = load_vec(bin_d, "bin_sb")
        bhn_sb = load_vec(bhn_d, "bhn_sb")
        colsum_sb = singles.tile([1, O], BF16)
        nc.sync.dma_start(colsum_sb[:], colsum_d)
        ones_col = singles.tile([P, 1], F32R)
        nc.sync.dma_start(ones_col[:], ones_col_d)
        ones_row = singles.tile([1, P], F32R)
        nc.sync.dma_start(ones_row[:], ones_row_d)
        eps_sb = singles.tile([1, 1], F32)
        nc.vector.memset(eps_sb[:], LN_EPS)

        new_hT_sb = [singles.tile([P, BL], BF16, tag=f"nh{ht}",
                                  name=f"nh{ht}") for ht in range(HT)]
        s_acc = [singles.tile([P, NF], F32R, tag=f"s_acc{bc}",
                              name=f"s_acc{bc}") for bc in range(NB)]
        q_acc = [singles.tile([P, NF], F32R, tag=f"q_acc{bc}",
                              name=f"q_acc{bc}") for bc in range(NB)]

        # readout weights: resident, split per-k and trickled in behind the
        # ramp-critical gate-weight loads (needed only in phase 2)
        woutF_sb = singles.tile([P, KT * O], BF16)

        # ---- LN stats helpers (used from inside phase 1 and in phase 2) ----
        nmu_row = {}
        rstd_row = {}
        rstd_bc = {}

        # PSUM bank chains for stats/broadcast are chosen so every WAR edge
        # points at a DVE op that executes early: gh0: s0 -> q1 -> pb1,
        # gh1: pb0 -> s1, gi0: q0 (freed by var0 long before group 4 reuses
        # it). None of pb0/pb1/s*/q* sit in the po rotation except gi0.
        stat_tags = {1: ("gh1", "gi1"), 0: ("gh1", "gi0")}
        pb_tags = {1: "gh0", 0: "gh1"}

        def emit_stats(bc):
            s_tag, q_tag = stat_tags[bc]
            psum_s = ps.tile([1, NF], F32, tag=s_tag, name=f"psum_s{bc}")
            nc.tensor.matmul(psum_s[:], ones_col[:], s_acc[bc][:],
                             start=True, stop=True)
            psum_q = ps.tile([1, NF], F32, tag=q_tag, name=f"psum_q{bc}")
            nc.tensor.matmul(psum_q[:], ones_col[:], q_acc[bc][:],
                             start=True, stop=True)

            nmu = rows.tile([1, NF], BF16, tag=f"nmu{bc}", name=f"nmu{bc}")
            nc.vector.tensor_scalar_mul(nmu[:], psum_s[:], -1.0 / H)
            nmu_row[bc] = nmu

            mu2 = gates.tile([1, NF], F32, tag="t", name=f"mu2_{bc}")
            nc.vector.tensor_mul(mu2[:], nmu[:], nmu[:])
            var = gates.tile([1, NF], F32, tag="u", name=f"var_{bc}")
            nc.vector.tensor_scalar_mul(var[:], psum_q[:], 1.0 / H)
            nc.vector.tensor_tensor(var[:], var[:], mu2[:],
                                    mybir.AluOpType.subtract)
            nc.scalar.activation(var[:], var[:],
                                 mybir.ActivationFunctionType.Sqrt,
                                 bias=eps_sb[:])
            rrow = rows.tile([1, NF], F32R, tag=f"rstd_row{bc}",
                             name=f"rstd{bc}")
            with nc.allow_low_precision(reason="f32r is fp32-width"):
                nc.vector.reciprocal(rrow[:], var[:])
            rstd_row[bc] = rrow

        def emit_pb(bc):
            pb = ps.tile([P, NF], F32, tag=pb_tags[bc], name=f"pb{bc}")
            nc.tensor.matmul(pb[:], ones_row[:], rstd_row[bc][:],
                             start=True, stop=True)
            rb = rows.tile([P, NF], F32, tag=f"rstd_bc{bc}",
                           name=f"rstd_bc{bc}")
            nc.vector.tensor_copy(rb[:], pb[:])
            rstd_bc[bc] = rb

        # ---- phase 1: gates + new_h -----------------------------------------
        for ht in range(HT):
            hs = slice(ht * P, (ht + 1) * P)
            if ht == 0:
                w_sb = w0
            else:
                w_sb = {g: wtile(g, ht) for g in XGATES + HGATES}
                for g in XGATES + HGATES:
                    load_w_flat(w_sb[g], g, ht)
            if 2 <= ht <= 5:
                for kk in range(2):
                    k = (ht - 2) * 2 + kk
                    nc.sync.dma_start(woutF_sb[:, k * O:(k + 1) * O],
                                      woutF_d[:, k * O:(k + 1) * O])

            pr = [ps.tile([P, NF], F32, tag=f"r{bc}", name=f"pr{bc}_{ht}")
                  for bc in range(NB)]
            pz = [ps.tile([P, NF], F32, tag=f"z{bc}", name=f"pz{bc}_{ht}")
                  for bc in range(NB)]
            pgi = [ps.tile([P, NF], F32, tag=f"gi{bc}", name=f"pgi{bc}_{ht}")
                   for bc in range(NB)]
            pgh = [ps.tile([P, NF], F32, tag=f"gh{bc}", name=f"pgh{bc}_{ht}")
                   for bc in range(NB)]

            def wk(g, k):
                return w_sb[g][:, k * P:(k + 1) * P]

            for bc in (range(NB) if ht < HT - 1 else (1, 0)):
                bs = slice(bc * NF, (bc + 1) * NF)
                # x-side chains first: the first half of the block only needs
                # the x tiles + x-side weights (DMA-matched ramp at ht=0)
                for k in range(KT):
                    nc.tensor.matmul(pr[bc][:], wk("ir", k), xb[bc][:, k, :],
                                     start=(k == 0), stop=False)
                for k in range(KT):
                    nc.tensor.matmul(pz[bc][:], wk("iz", k), xb[bc][:, k, :],
                                     start=(k == 0), stop=False)
                for k in range(KT):
                    nc.tensor.matmul(pgi[bc][:], wk("in", k), xb[bc][:, k, :],
                                     start=(k == 0), stop=(k == KT - 1))
                if ht == HT - 1 and bc == 0:
                    # bc1 LN stats (bc1 ran first at ht7): reduce matmuls +
                    # rstd chain run in the shadow of this block's matmuls
                    emit_stats(1)
                # h-side
                for k in range(KT):
                    nc.tensor.matmul(pr[bc][:], wk("hr", k), hb[bc][:, k, :],
                                     start=False, stop=(k == KT - 1))
                r_sb = gates.tile([P, NF], F32, tag="r_act")
                nc.scalar.activation(r_sb[:], pr[bc][:],
                                     mybir.ActivationFunctionType.Sigmoid,
                                     bias=bir_sb[:, ht:ht + 1])
                for k in range(KT):
                    nc.tensor.matmul(pz[bc][:], wk("hz", k), hb[bc][:, k, :],
                                     start=False, stop=(k == KT - 1))
                z_sb = gates.tile([P, NF], F32, tag="z_act")
                nc.scalar.activation(z_sb[:], pz[bc][:],
                                     mybir.ActivationFunctionType.Sigmoid,
                                     bias=biz_sb[:, ht:ht + 1])
                for k in range(KT):
                    nc.tensor.matmul(pgh[bc][:], wk("hn", k), hb[bc][:, k, :],
                                     start=(k == 0), stop=(k == KT - 1))

                # epilogue
                t_sb = gates.tile([P, NF], F32, tag="t")
                nc.vector.scalar_tensor_tensor(t_sb[:], pgh[bc][:],
                                               bhn_sb[:, ht:ht + 1], r_sb[:],
                                               mybir.AluOpType.add,
                                               mybir.AluOpType.mult)
                nc.vector.tensor_add(t_sb[:], t_sb[:], pgi[bc][:])
                n_sb = gates.tile([P, NF], F32, tag="r_act", name="n_sb")
                nc.scalar.activation(n_sb[:], t_sb[:],
                                     mybir.ActivationFunctionType.Tanh,
                                     bias=bin_sb[:, ht:ht + 1])

                u_sb = gates.tile([P, NF], F32, tag="u")
                nc.vector.tensor_tensor(u_sb[:], hb[bc][:, ht, :], n_sb[:],
                                        mybir.AluOpType.subtract)
                nc.vector.tensor_mul(u_sb[:], z_sb[:], u_sb[:])
                nh = new_hT_sb[ht][:, bs]
                nc.vector.tensor_add(nh, n_sb[:], u_sb[:])

                # LN stat partials: elementwise accumulate over h-tiles (DVE),
                # cross-partition reduce later via a ones-column matmul.
                sq_sb = gates.tile([P, NF], F32R, tag="t", name="sq_sb")
                if ht == 0:
                    nc.vector.tensor_copy(s_acc[bc][:], nh)
                    nc.scalar.activation(q_acc[bc][:], nh,
                                         mybir.ActivationFunctionType.Square)
                else:
                    nc.vector.tensor_tensor(s_acc[bc][:],
                                            s_acc[bc][:].bitcast(F32), nh,
                                            mybir.AluOpType.add)
                    nc.scalar.activation(sq_sb[:], nh,
                                         mybir.ActivationFunctionType.Square)
                    nc.vector.tensor_tensor(q_acc[bc][:],
                                            q_acc[bc][:].bitcast(F32),
                                            sq_sb[:].bitcast(F32),
                                            mybir.AluOpType.add)

            # one store per ht (fewer, larger DMA triggers); the gpsimd ring
            # has finished its input loads well before the first store
            nc.gpsimd.dma_start(nhT_d[hs, :], new_hT_sb[ht][:])

        # ---- phase 2: LN scale factors + readout ----------------------------
        # The bc0 stats chain was emitted mid-way through ht7-bc1's matmul
        # block, so rstd0 (including the 3.3us reciprocal) is ready before
    # phase 2 begins; bc1 stats follow under the first readout groups.

        po_tags = ("r0", "z0", "r1", "z1", "gi0", "gi1")
        PIPE = 5
        groups = [(ot, bc) for bc in (1, 0) for ot in range(OT)]
        pending = {}

        def finalize(i):
            ot, bc = groups[i]
            po = pending.pop(i)
            os_ = slice(ot * P, (ot + 1) * P)
            bs = slice(bc * NF, (bc + 1) * NF)
            o_sb = gates.tile([P, NF], BF16, tag=("t", "u", "z_act")[i % 3],
                              name=f"o_{ot}_{bc}")
            nc.vector.tensor_mul(o_sb[:], po[:], rstd_bc[bc][:])
            nc.scalar.dma_start(outT_d[os_, bs], o_sb[:])

        done = 0
        for i, (ot, bc) in enumerate(groups):
            bs = slice(bc * NF, (bc + 1) * NF)
            os_ = slice(ot * P, (ot + 1) * P)
            po = ps.tile([P, NF], F32, tag=po_tags[i % len(po_tags)],
                         name=f"po_{ot}_{bc}")
            for k in range(HT):
                nc.tensor.matmul(po[:], woutF_sb[:, k * O + ot * P:
                                                 k * O + (ot + 1) * P],
                                 new_hT_sb[k][:, bs],
                                 start=(k == 0), stop=False)
            # -= mu[b] * colsum[o]  (rank-1, K=1) closes the accumulation
            nc.tensor.matmul(po[:], colsum_sb[0:1, os_], nmu_row[bc][:],
                             start=False, stop=True)
            pending[i] = po
            if i == 1:
                emit_pb(1)
            elif i == 3:
                emit_stats(0)
            if i >= PIPE:
                finalize(done)
                done += 1
            if i == 7:
                emit_pb(0)
            if i >= 10 and done <= i - 1:
                # drain the pipeline early so the tail is short
                finalize(done)
                done += 1
        while done < len(groups):
            finalize(done)
            done += 1

    nc.compile()
    return nc


def _to_bf16(a):
    return np.ascontiguousarray(np.asarray(a, np.float32)).astype(BF16NP)


def _w_layout(W):
    # [p, ht*KT*P + k*P + j] = W[k*P+p, ht*P+j]
    W = np.asarray(W, np.float32)
    return np.ascontiguousarray(
        W.reshape(KT, P, HT, P).transpose(1, 2, 0, 3).reshape(P, HT * KT * P)
    ).astype(BF16NP)


def _bias_layout(b):
    return np.ascontiguousarray(np.asarray(b, np.float32).reshape(HT, P).T)


def kernel(x, h, Wir, bir, Wiz, biz, Win, bin_, Whr, Whz, Whn, bhn,
           ln_scale, ln_bias, Wout, bout):
    global _COMPILED, LAST_RES
    if _COMPILED is None:
        _COMPILED = _build()
    nc = _COMPILED

    x = np.asarray(x, np.float32)
    h = np.asarray(h, np.float32)
    xT = np.ascontiguousarray(x.T).astype(BF16NP)
    hT = np.ascontiguousarray(h.T).astype(BF16NP)
    Wout = np.asarray(Wout, np.float32)
    ln_scale = np.asarray(ln_scale, np.float32)
    ln_bias = np.asarray(ln_bias, np.float32)
    woutF = ln_scale[:, None] * Wout
    woutF_l = np.ascontiguousarray(
        woutF.reshape(KT, P, O).transpose(1, 0, 2).reshape(P, KT * O)
    ).astype(BF16NP)
    boutF = (np.asarray(bout, np.float32) + ln_bias @ Wout).astype(np.float32)
    colsum = (ln_scale @ Wout).reshape(1, O).astype(BF16NP)

    common = {
        "Wir": _w_layout(Wir), "Wiz": _w_layout(Wiz), "Win": _w_layout(Win),
        "Whr": _w_layout(Whr), "Whz": _w_layout(Whz), "Whn": _w_layout(Whn),
        "woutF": woutF_l,
        "bir": _bias_layout(bir), "biz": _bias_layout(biz),
        "bin": _bias_layout(bin_), "bhn": _bias_layout(bhn),
        "colsum": colsum,
        "ones_col": np.ones((P, 1), np.float32),
        "ones_row": np.ones((1, P), np.float32),
    }
    in_maps = []
    for c in range(NCORES):
        bsl = slice(c * BL, (c + 1) * BL)
        in_maps.append({
            **common,
            "xT": np.ascontiguousarray(xT[:, bsl]),
            "hT": np.ascontiguousarray(hT[:, bsl]),
        })

    res = bass_utils.run_bass_kernel_spmd(nc, in_maps,
                                          core_ids=list(range(NCORES)),
                                          trace=TRACE)
    LAST_RES = res
    new_hT = np.concatenate([res.results[c]["nhT"] for c in range(NCORES)],
                            axis=1)
    outT = np.concatenate([res.results[c]["outT"] for c in range(NCORES)],
                          axis=1)
    new_h = np.ascontiguousarray(new_hT.T).astype(np.float32)
    out = np.ascontiguousarray(outT.T).astype(np.float32) + boutF[None, :]
    return new_h, out
